# revision 2
# baseline (speedup 1.0000x reference)
"""Trainium2 Bass kernel for the batched peg-solitaire env step.

Data-parallel over 8 NeuronCores: each core processes 32768 envs.

Per-core layout: envs tiled 128-per-partition-tile (env = t*128 + p).
Per tile:
  - pos/mid/tgt action-table gathers as fused compare-multiply-accumulate
    (scalar_tensor_tensor with accum_out) against an iota row.
  - peg update via iota-compare chains (no scatter needed).
  - feasibility: one TensorE matmul of transposed new-pegs against a
    constant count matrix; min over 132 actions == 0 <=> a move exists.
  - board scatter for states ch0 rides in the same matmul (49 extra cols).
  - states ch1/ch2 from a per-env ratio scalar against constant masks.
"""

import numpy as np

N_CORES = 8
N_ENV = 262144
P = 128
E = N_ENV // N_CORES
NT = E // P
NA = 132
NB = 33

_CACHE = {}


def _build_bass():
    import concourse.bacc as bacc
    import concourse.mybir as mybir
    from concourse import masks
    from concourse.tile import TileContext

    f32 = mybir.dt.float32
    i32 = mybir.dt.int32
    u8 = mybir.dt.uint8
    Alu = mybir.AluOpType

    nc = bacc.Bacc(None)

    pegs = nc.dram_tensor("pegs", [E, NB], f32, kind="ExternalInput")
    n_pegs = nc.dram_tensor("n_pegs", [E], i32, kind="ExternalInput")
    done = nc.dram_tensor("done", [E], u8, kind="ExternalInput")
    trew = nc.dram_tensor("total_reward", [E], f32, kind="ExternalInput")
    actions = nc.dram_tensor("actions", [E], i32, kind="ExternalInput")
    tbl = nc.dram_tensor("tbl", [P, 3 * NA], f32, kind="ExternalInput")
    rhs2 = nc.dram_tensor("rhs2", [NB + 1, NA + 49], f32, kind="ExternalInput")
    acst = nc.dram_tensor("acst", [P, 98], f32, kind="ExternalInput")
    bcst = nc.dram_tensor("bcst", [P, 98], f32, kind="ExternalInput")

    rewards_o = nc.dram_tensor("rewards_o", [E], f32, kind="ExternalOutput")
    states_o = nc.dram_tensor("states_o", [E, 147], f32, kind="ExternalOutput")
    done_o = nc.dram_tensor("done_o", [E], u8, kind="ExternalOutput")
    pegs_o = nc.dram_tensor("pegs_o", [E, NB], f32, kind="ExternalOutput")
    npegs_o = nc.dram_tensor("npegs_o", [E], i32, kind="ExternalOutput")
    trew_o = nc.dram_tensor("trew_o", [E], f32, kind="ExternalOutput")

    with TileContext(nc) as tc:
        with (
            tc.tile_pool(name="const", bufs=1) as constp,
            tc.tile_pool(name="small", bufs=1) as smallp,
            tc.tile_pool(name="work", bufs=3) as workp,
            tc.tile_pool(name="psT", bufs=2, space="PSUM") as psTp,
            tc.tile_pool(name="ps2", bufs=2, space="PSUM") as ps2p,
        ):
            # ---- constants ----
            ident = constp.tile([P, P], f32)
            masks.make_identity(nc, ident[:])
            tbl_sb = constp.tile([P, 3 * NA], f32)
            nc.sync.dma_start(tbl_sb[:], tbl[:, :])
            rhs2_sb = constp.tile([NB + 1, NA + 49], f32)
            nc.sync.dma_start(rhs2_sb[:], rhs2[:, :])
            a_sb = constp.tile([P, 98], f32)
            nc.sync.dma_start(a_sb[:], acst[:, :])
            b_sb = constp.tile([P, 98], f32)
            nc.sync.dma_start(b_sb[:], bcst[:, :])
            a3 = a_sb.rearrange("p (c k) -> p c k", k=2)
            b3 = b_sb.rearrange("p (c k) -> p c k", k=2)
            iota_i = constp.tile([P, NA], i32)
            nc.gpsimd.iota(iota_i[:], pattern=[[1, NA]], base=0, channel_multiplier=0)
            iota_f = constp.tile([P, NA], f32)
            nc.vector.tensor_copy(iota_f[:], iota_i[:])
            i33 = iota_f[:, :NB]

            # ---- small tensors, column layout (128, NT): env = t*128 + p ----
            npg_i = smallp.tile([P, NT], i32)
            nc.sync.dma_start(npg_i[:], n_pegs.rearrange("(t p) -> p t", p=P))
            done_u = smallp.tile([P, NT], u8)
            nc.sync.dma_start(done_u[:], done.rearrange("(t p) -> p t", p=P))
            trew_f = smallp.tile([P, NT], f32)
            nc.sync.dma_start(trew_f[:], trew.rearrange("(t p) -> p t", p=P))
            act_i = smallp.tile([P, NT], i32)
            nc.sync.dma_start(act_i[:], actions.rearrange("(t p) -> p t", p=P))

            act_f = smallp.tile([P, NT], f32)
            nc.vector.tensor_copy(act_f[:], act_i[:])
            npf = smallp.tile([P, NT], f32)
            nc.vector.tensor_copy(npf[:], npg_i[:])
            done_f = smallp.tile([P, NT], f32)
            nc.vector.tensor_copy(done_f[:], done_u[:])

            # pr = (npf - 2)/31 ; iseq = (npf == 2) ; rewards = 1/31 + iseq*30/31
            pr = smallp.tile([P, NT], f32)
            nc.vector.tensor_scalar(
                pr[:], npf[:], 1.0 / 31.0, -2.0 / 31.0, op0=Alu.mult, op1=Alu.add
            )
            iseq = smallp.tile([P, NT], f32)
            nc.vector.tensor_scalar(iseq[:], npf[:], 2.0, None, op0=Alu.is_equal)
            rew = smallp.tile([P, NT], f32)
            nc.vector.tensor_scalar(
                rew[:], iseq[:], 30.0 / 31.0, 1.0 / 31.0, op0=Alu.mult, op1=Alu.add
            )
            trew_n = smallp.tile([P, NT], f32)
            nc.vector.tensor_tensor(trew_n[:], trew_f[:], rew[:], op=Alu.add)
            npf_n = smallp.tile([P, NT], f32)
            nc.vector.tensor_scalar(npf_n[:], npf[:], -1.0, None, op0=Alu.add)
            npg_out = smallp.tile([P, NT], i32)
            nc.vector.tensor_copy(npg_out[:], npf_n[:])
            feas = smallp.tile([P, NT], f32)

            # ---- per-tile loop ----
            for t in range(NT):
                pegs_t = workp.tile([P, NB], f32, tag="pegs")
                nc.sync.dma_start(pegs_t[:], pegs[t * P : (t + 1) * P, :])

                act_s = act_f[:, t : t + 1]
                dump = workp.tile([P, NA], f32, tag="dump")
                pmt = workp.tile([P, 3], f32, tag="pmt")
                for k in range(3):
                    nc.vector.scalar_tensor_tensor(
                        dump[:],
                        iota_f[:],
                        act_s,
                        tbl_sb[:, k * NA : (k + 1) * NA],
                        op0=Alu.is_equal,
                        op1=Alu.mult,
                        accum_out=pmt[:, k : k + 1],
                    )

                s1 = workp.tile([P, NB], f32, tag="s1")
                s2 = workp.tile([P, NB], f32, tag="s2")
                np_t = workp.tile([P, NB + 1], f32, tag="np")
                # new_pegs = (i!=pos)*(i!=mid)*(i!=tgt)*pegs + (i==tgt)
                nc.vector.scalar_tensor_tensor(
                    s1[:], i33, pmt[:, 0:1], pegs_t[:], op0=Alu.not_equal, op1=Alu.mult
                )
                nc.vector.scalar_tensor_tensor(
                    s2[:], i33, pmt[:, 1:2], s1[:], op0=Alu.not_equal, op1=Alu.mult
                )
                nc.vector.scalar_tensor_tensor(
                    s1[:], i33, pmt[:, 2:3], s2[:], op0=Alu.not_equal, op1=Alu.mult
                )
                nc.vector.scalar_tensor_tensor(
                    np_t[:, :NB], i33, pmt[:, 2:3], s1[:], op0=Alu.is_equal, op1=Alu.add
                )
                nc.vector.memset(np_t[:, NB : NB + 1], 1.0)

                psT = psTp.tile([NB + 1, P], f32)
                nc.tensor.transpose(psT[:], np_t[:], ident[:])
                lhsT = workp.tile([NB + 1, P], f32, tag="lhsT")
                nc.scalar.copy(lhsT[:], psT[:])

                ps2 = ps2p.tile([P, NA + 49], f32)
                nc.tensor.matmul(ps2[:], lhsT[:], rhs2_sb[:])

                nc.vector.tensor_reduce(
                    feas[:, t : t + 1],
                    ps2[:, :NA],
                    axis=mybir.AxisListType.X,
                    op=Alu.min,
                )

                st = workp.tile([P, 147], f32, tag="st")
                st3 = st.rearrange("p (c k) -> p c k", k=3)
                nc.vector.scalar_tensor_tensor(
                    st3[:, :, 1:3],
                    a3,
                    pr[:, t : t + 1],
                    b3,
                    op0=Alu.mult,
                    op1=Alu.add,
                )
                nc.scalar.copy(
                    st3[:, :, 0:1],
                    ps2[:, NA : NA + 49].rearrange("p (c k) -> p c k", k=1),
                )

                nc.sync.dma_start(states_o[t * P : (t + 1) * P, :], st[:])
                nc.sync.dma_start(pegs_o[t * P : (t + 1) * P, :], np_t[:, :NB])

            # ---- epilogue: done_new = max(iseq, done_f, min(feas,1)) ----
            dminc = smallp.tile([P, NT], f32)
            nc.vector.tensor_scalar(dminc[:], feas[:], 1.0, None, op0=Alu.min)
            t1 = smallp.tile([P, NT], f32)
            nc.vector.tensor_tensor(t1[:], iseq[:], done_f[:], op=Alu.max)
            dnew = smallp.tile([P, NT], f32)
            nc.vector.tensor_tensor(dnew[:], t1[:], dminc[:], op=Alu.max)
            dnew_u = smallp.tile([P, NT], u8)
            nc.vector.tensor_copy(dnew_u[:], dnew[:])

            nc.sync.dma_start(rewards_o.rearrange("(t p) -> p t", p=P), rew[:])
            nc.sync.dma_start(done_o.rearrange("(t p) -> p t", p=P), dnew_u[:])
            nc.sync.dma_start(npegs_o.rearrange("(t p) -> p t", p=P), npg_out[:])
            nc.sync.dma_start(trew_o.rearrange("(t p) -> p t", p=P), trew_n[:])

    nc.finalize()
    return nc


def _consts(action_pos_ids, action_mid_indices, action_target_indices, oob_mask,
            i_indices, j_indices):
    Pa = np.asarray(action_pos_ids, np.int64)
    Ma = np.asarray(action_mid_indices, np.int64)
    Ta = np.asarray(action_target_indices, np.int64)
    oob = np.asarray(oob_mask, np.bool_)
    ii = np.asarray(i_indices, np.int64)
    jj = np.asarray(j_indices, np.int64)

    tbl = np.zeros((P, 3 * NA), np.float32)
    tbl[:, 0:NA] = Pa[None, :]
    tbl[:, NA : 2 * NA] = Ma[None, :]
    tbl[:, 2 * NA : 3 * NA] = Ta[None, :]

    rhs2 = np.zeros((NB + 1, NA + 49), np.float32)
    # count2[a] = 2 + 3*oob[a] - np[P[a]] - np[M[a]] + np[T[a]]
    np.add.at(rhs2, (Pa, np.arange(NA)), -1.0)
    np.add.at(rhs2, (Ma, np.arange(NA)), -1.0)
    np.add.at(rhs2, (Ta, np.arange(NA)), 1.0)
    rhs2[NB, :NA] = 2.0 + 3.0 * oob.astype(np.float32)
    # board scatter: col NA+b gets cell c where 7*i+j == b
    cell = 7 * ii + jj
    rhs2[np.arange(NB), NA + cell] = 1.0

    acst = np.zeros((P, 98), np.float32)
    bcst = np.zeros((P, 98), np.float32)
    acst[:, 0::2] = 1.0
    acst[:, 1::2] = -1.0
    bcst[:, 1::2] = 1.0
    return tbl, rhs2, acst, bcst


def kernel(pegs, n_pegs, done, total_reward, actions,
           action_pos_ids, action_mid_indices, action_target_indices,
           oob_mask, i_indices, j_indices):
    from concourse.bass_utils import run_bass_kernel_spmd

    if "nc" not in _CACHE:
        _CACHE["nc"] = _build_bass()
    nc = _CACHE["nc"]

    tbl, rhs2, acst, bcst = _consts(
        action_pos_ids, action_mid_indices, action_target_indices,
        oob_mask, i_indices, j_indices)

    pegs = np.ascontiguousarray(np.asarray(pegs, np.float32))
    n_pegs = np.ascontiguousarray(np.asarray(n_pegs, np.int32))
    done_u = np.ascontiguousarray(np.asarray(done).astype(np.uint8))
    trew = np.ascontiguousarray(np.asarray(total_reward, np.float32))
    actions = np.ascontiguousarray(np.asarray(actions, np.int32))

    in_maps = []
    for c in range(N_CORES):
        s = slice(c * E, (c + 1) * E)
        in_maps.append({
            "pegs": pegs[s],
            "n_pegs": n_pegs[s],
            "done": done_u[s],
            "total_reward": trew[s],
            "actions": actions[s],
            "tbl": tbl,
            "rhs2": rhs2,
            "acst": acst,
            "bcst": bcst,
        })

    res = run_bass_kernel_spmd(nc, in_maps, core_ids=list(range(N_CORES)))

    rewards = np.concatenate([res.results[c]["rewards_o"] for c in range(N_CORES)])
    states = np.concatenate([res.results[c]["states_o"] for c in range(N_CORES)])
    done_new = np.concatenate([res.results[c]["done_o"] for c in range(N_CORES)])
    pegs_new = np.concatenate([res.results[c]["pegs_o"] for c in range(N_CORES)])
    npegs_new = np.concatenate([res.results[c]["npegs_o"] for c in range(N_CORES)])
    trew_new = np.concatenate([res.results[c]["trew_o"] for c in range(N_CORES)])

    n = N_ENV
    return (
        rewards.astype(np.float32),
        states.reshape(n, 7, 7, 3).astype(np.float32),
        done_new.astype(bool),
        pegs_new.astype(np.float32),
        npegs_new.astype(np.int32),
        trew_new.astype(np.float32),
    )


# revision 23
# speedup vs baseline: 28289.6809x; 28289.6809x over previous
"""Trainium2 Bass kernel for the batched peg-solitaire env step.

Data-parallel over 8 NeuronCores: each core processes 32768 envs.

Architecture (per core, env = t*128 + p, supertiles of G=4 tiles):
  - action one-hot built TRANSPOSED (actions on partitions, envs on free):
    a broadcast matmul computes act[j]-m and act[j]-128-(m%4); one wide
    is_equal gives OH (128, 2*512). Actions >= 128 are covered by the
    duplicated-row chunk with tables scaled by 1/32.
  - stage-1 matmuls (OH chunks vs 66-wide tables) give union/h_tgt per env.
  - peg update: np = (union==0)*pegs + h_tgt  (2 vector ops).
  - stage-2: TensorE transpose of np(+ones col), matmul vs a (34,181)
    constant -> 132 feasibility counts + 49-cell board scatter in PSUM.
  - feasibility: Relu(1-count) with fused accumulate on ScalarE ->
    #feasible; ==0 joins the done computation.
  - states ch1/ch2 computed on GPSIMD from a per-env ratio scalar.
"""

import numpy as np

N_CORES = 8
N_ENV = 262144
P = 128
E = N_ENV // N_CORES
NT = E // P
NA = 132
NB = 33

_CACHE = {}


def _build_bass(repeat=1, pool_free=True):
    import concourse.bacc as bacc
    import concourse.mybir as mybir
    from concourse import masks
    from concourse.tile import TileContext

    f32 = mybir.dt.float32
    i32 = mybir.dt.int32
    u8 = mybir.dt.uint8
    bf16 = mybir.dt.bfloat16
    Alu = mybir.AluOpType
    Act = mybir.ActivationFunctionType

    nc = bacc.Bacc(None)

    pegs = nc.dram_tensor("pegs", [E, NB], f32, kind="ExternalInput")
    n_pegs = nc.dram_tensor("n_pegs", [E], i32, kind="ExternalInput")
    done = nc.dram_tensor("done", [E], u8, kind="ExternalInput")
    trew = nc.dram_tensor("total_reward", [E], f32, kind="ExternalInput")
    actions = nc.dram_tensor("actions", [E], i32, kind="ExternalInput")
    u1 = nc.dram_tensor("u1", [P, 66], bf16, kind="ExternalInput")
    u2 = nc.dram_tensor("u2", [P, 66], bf16, kind="ExternalInput")
    rhs2 = nc.dram_tensor("rhs2", [NB + 1, NA + 49], bf16, kind="ExternalInput")
    acst = nc.dram_tensor("acst", [P, 98], f32, kind="ExternalInput")
    bcst = nc.dram_tensor("bcst", [P, 98], f32, kind="ExternalInput")
    e8 = nc.dram_tensor("e8", [8, 8 * P], bf16, kind="ExternalInput")
    mcol = nc.dram_tensor("mcol", [P, 2], f32, kind="ExternalInput")

    rewards_o = nc.dram_tensor("rewards_o", [E], f32, kind="ExternalOutput")
    states_o = nc.dram_tensor("states_o", [E, 147], f32, kind="ExternalOutput")
    done_o = nc.dram_tensor("done_o", [E], u8, kind="ExternalOutput")
    pegs_o = nc.dram_tensor("pegs_o", [E, NB], bf16, kind="ExternalOutput")
    npegs_o = nc.dram_tensor("npegs_o", [E], i32, kind="ExternalOutput")
    trew_o = nc.dram_tensor("trew_o", [E], f32, kind="ExternalOutput")

    G = 4
    S = G * P  # envs per supertile

    with TileContext(nc) as tc:
        with (
            tc.tile_pool(name="const", bufs=1) as constp,
            tc.tile_pool(name="small", bufs=1) as smallp,
            tc.tile_pool(name="work", bufs=6) as workp,
            tc.tile_pool(name="oh", bufs=3) as ohp,
            tc.tile_pool(name="psab", bufs=1, space="PSUM") as psabp,
            tc.tile_pool(name="ps1", bufs=2, space="PSUM") as ps1p,
            tc.tile_pool(name="psT", bufs=1, space="PSUM") as psTp,
            tc.tile_pool(name="ps2", bufs=2, space="PSUM") as ps2p,
        ):
            # ---- constants ----
            ident = constp.tile([P, P], bf16)
            masks.make_identity(nc, ident[:])
            u1_sb = constp.tile([P, 66], bf16)
            nc.sync.dma_start(u1_sb[:], u1[:, :])
            u2_sb = constp.tile([P, 66], bf16)
            nc.sync.dma_start(u2_sb[:], u2[:, :])
            rhs2_sb = constp.tile([NB + 1, NA + 49], bf16)
            nc.sync.dma_start(rhs2_sb[:], rhs2[:, :])
            a_sb = constp.tile([P, 98], f32)
            nc.sync.dma_start(a_sb[:], acst[:, :])
            b_sb = constp.tile([P, 98], f32)
            nc.sync.dma_start(b_sb[:], bcst[:, :])
            a3 = a_sb.rearrange("p (c k) -> p c k", k=2)
            b3 = b_sb.rearrange("p (c k) -> p c k", k=2)

            # broadcast-matmul selector (row q -> all 128 out partitions)
            # and per-partition compare scalars [m | 128+m%4]
            e8_sb = constp.tile([8, 8 * P], bf16)
            nc.sync.dma_start(e8_sb[:], e8[:, :])
            mcol_sb = constp.tile([P, 2], f32)
            nc.sync.dma_start(mcol_sb[:], mcol[:, :])

            # actions as f32 rows: partitions 0-7, one chunk per partition
            QW = E // 8
            t8 = constp.tile([8, QW], bf16)
            t8i = constp.tile([8, QW], i32)
            nc.sync.dma_start(t8i[:], actions.rearrange("(q i) -> q i", i=QW))
            nc.vector.tensor_copy(t8[:], t8i[:])

            # ---- small tensors, column layout (128, NT): env = t*128 + p ----
            npg_i = smallp.tile([P, NT], i32)
            nc.sync.dma_start(npg_i[:], n_pegs.rearrange("(t p) -> p t", p=P))
            done_u = smallp.tile([P, NT], u8)
            nc.sync.dma_start(done_u[:], done.rearrange("(t p) -> p t", p=P))
            trew_f = smallp.tile([P, NT], f32)
            nc.sync.dma_start(trew_f[:], trew.rearrange("(t p) -> p t", p=P))

            npf = smallp.tile([P, NT], f32)
            nc.vector.tensor_copy(npf[:], npg_i[:])
            done_f = smallp.tile([P, NT], f32)
            nc.vector.tensor_copy(done_f[:], done_u[:])

            # pr = (npf - 2)/31 ; iseq = (npf == 2) ; rewards = 1/31 + iseq*30/31
            pr = smallp.tile([P, NT], f32)
            nc.vector.tensor_scalar(
                pr[:], npf[:], 1.0 / 31.0, -2.0 / 31.0, op0=Alu.mult, op1=Alu.add
            )
            iseq = smallp.tile([P, NT], f32)
            nc.vector.tensor_scalar(iseq[:], npf[:], 2.0, None, op0=Alu.is_equal)
            rew = smallp.tile([P, NT], f32)
            nc.vector.tensor_scalar(
                rew[:], iseq[:], 30.0 / 31.0, 1.0 / 31.0, op0=Alu.mult, op1=Alu.add
            )
            trew_n = smallp.tile([P, NT], f32)
            nc.vector.tensor_tensor(trew_n[:], trew_f[:], rew[:], op=Alu.add)
            npf_n = smallp.tile([P, NT], f32)
            nc.vector.tensor_scalar(npf_n[:], npf[:], -1.0, None, op0=Alu.add)
            npg_out = smallp.tile([P, NT], i32)
            nc.vector.tensor_copy(npg_out[:], npf_n[:])
            fmin = smallp.tile([P, NT], f32)

            # ---- supertile loop ----
            pegs_v = pegs.rearrange("(a p) c -> p a c", p=P)
            states_v = states_o.rearrange("(a p) c -> p a c", p=P)
            pegs_o_v = pegs_o.rearrange("(a p) c -> p a c", p=P)
            for s in [ss for _ in range(repeat) for ss in range(E // S)]:
                t0 = s * G
                e0 = s * S
                q, off = e0 // QW, e0 % QW
                h = s % 2

                if h == 0:
                    pegs_big2 = workp.tile([P, 2 * G, NB], f32, tag="pegs")
                    nc.sync.dma_start(
                        pegs_big2[:], pegs_v[:, t0 : t0 + 2 * G, :]
                    )
                    np_big2 = workp.tile([P, 2 * G, NB + 1], bf16, tag="np")
                    if pool_free:
                        nc.vector.memset(np_big2[:, :, NB : NB + 1], 1.0)
                    else:
                        nc.gpsimd.memset(np_big2[:, :, NB : NB + 1], 1.0)
                    st_big2 = workp.tile([P, 2 * G, 147], f32, tag="st")
                pegs_big = pegs_big2[:, h * G : (h + 1) * G, :]
                np_big = np_big2[:, h * G : (h + 1) * G, :]
                st_big = st_big2[:, h * G : (h + 1) * G, :]

                # OH (transposed): rows m: (act==m) | rows m: (act==128+m%4)
                psab = psabp.tile([P, S], f32)
                nc.tensor.matmul(
                    psab[:], e8_sb[:, q * P : (q + 1) * P], t8[:, off : off + S]
                )
                psab_sb = ohp.tile([P, S], bf16, tag="psab_sb")
                nc.scalar.copy(psab_sb[:], psab[:])
                oh = ohp.tile([P, 2 * S], bf16, tag="oh")
                nc.vector.tensor_scalar(
                    oh[:, 0:S], psab_sb[:], mcol_sb[:, 0:1], None, op0=Alu.is_equal
                )
                nc.vector.tensor_scalar(
                    oh[:, S : 2 * S], psab_sb[:], mcol_sb[:, 1:2], None, op0=Alu.is_equal
                )

                ps1 = ps1p.tile([P, G, 66], f32)
                psT = psTp.tile([NB + 1, S], bf16)
                ps2 = ps2p.tile([P, G, 256], f32)
                lhsT = workp.tile([NB + 1, S], bf16, tag="lhsT")

                for g in range(G):
                    # stage-1: union | h_tgt for this tile's 128 envs
                    nc.tensor.matmul(
                        ps1[:, g, :], oh[:, g * P : (g + 1) * P], u1_sb[:],
                        start=True, stop=False,
                    )
                    nc.tensor.matmul(
                        ps1[:, g, :], oh[:, S + g * P : S + (g + 1) * P], u2_sb[:],
                        start=False, stop=True,
                    )
                # np = (union==0)*pegs + h_tgt   (batched over all G tiles)
                nc.vector.scalar_tensor_tensor(
                    np_big[:, :, :NB],
                    ps1[:, :, 0:NB],
                    0.0,
                    pegs_big,
                    op0=Alu.is_equal,
                    op1=Alu.mult,
                )
                nc.vector.tensor_tensor(
                    np_big[:, :, :NB],
                    np_big[:, :, :NB],
                    ps1[:, :, NB:66],
                    op=Alu.add,
                )
                for g in range(G):
                    nc.tensor.transpose(
                        psT[:, g * P : (g + 1) * P], np_big[:, g, :], ident[:]
                    )

                nc.scalar.copy(lhsT[:], psT[:])

                for g in range(G):
                    nc.tensor.matmul(
                        ps2[:, g, : NA + 49], lhsT[:, g * P : (g + 1) * P], rhs2_sb[:]
                    )
                # min over the 132 feasibility counts, all G tiles at once
                nc.vector.tensor_reduce(
                    fmin[:, t0 : t0 + G],
                    ps2[:, :, 0:NA],
                    axis=mybir.AxisListType.X,
                    op=Alu.min,
                )
                # board scatter -> states ch0, all G tiles at once
                st4 = st_big.rearrange("p g (c k) -> p g c k", k=3)
                nc.scalar.copy(
                    st4[:, :, :, 0:1],
                    ps2[:, :, NA : NA + 49].rearrange("p g (c k) -> p g c k", k=1),
                )
                for g in range(G):
                    t = t0 + g
                    st3 = st_big[:, g, :].rearrange("p (c k) -> p c k", k=3)
                    # ch1 = pr, ch2 = 1-pr  via A*pr + B
                    if pool_free:
                        nc.vector.scalar_tensor_tensor(
                            st3[:, :, 1:3], a3, pr[:, t : t + 1], b3,
                            op0=Alu.mult, op1=Alu.add,
                        )
                    else:
                        ctmp = workp.tile([P, 98], f32, tag="ctmp")
                        ctmp3 = ctmp.rearrange("p (c k) -> p c k", k=2)
                        nc.gpsimd.tensor_scalar(
                            ctmp[:], a_sb[:], pr[:, t : t + 1], None, op0=Alu.mult
                        )
                        nc.gpsimd.tensor_tensor(
                            st3[:, :, 1:3], ctmp3, b3, op=Alu.add
                        )

                if h == 1:
                    nc.sync.dma_start(
                        states_v[:, t0 - G : t0 + G, :], st_big2[:]
                    )
                    nc.sync.dma_start(
                        pegs_o_v[:, t0 - G : t0 + G, :], np_big2[:, :, :NB]
                    )

            # ---- epilogue: done_new = max(iseq, done_f, feas==0) ----
            dminc = smallp.tile([P, NT], f32)
            nc.vector.tensor_scalar(dminc[:], fmin[:], 1.0, None, op0=Alu.min)
            t1 = smallp.tile([P, NT], f32)
            nc.vector.tensor_tensor(t1[:], iseq[:], done_f[:], op=Alu.max)
            dnew = smallp.tile([P, NT], f32)
            nc.vector.tensor_tensor(dnew[:], t1[:], dminc[:], op=Alu.max)
            dnew_u = smallp.tile([P, NT], u8)
            nc.vector.tensor_copy(dnew_u[:], dnew[:])

            nc.sync.dma_start(rewards_o.rearrange("(t p) -> p t", p=P), rew[:])
            nc.sync.dma_start(done_o.rearrange("(t p) -> p t", p=P), dnew_u[:])
            nc.sync.dma_start(npegs_o.rearrange("(t p) -> p t", p=P), npg_out[:])
            nc.sync.dma_start(trew_o.rearrange("(t p) -> p t", p=P), trew_n[:])

    nc.finalize()
    return nc


def _consts(action_pos_ids, action_mid_indices, action_target_indices, oob_mask,
            i_indices, j_indices):
    Pa = np.asarray(action_pos_ids, np.int64)
    Ma = np.asarray(action_mid_indices, np.int64)
    Ta = np.asarray(action_target_indices, np.int64)
    oob = np.asarray(oob_mask, np.bool_)
    ii = np.asarray(i_indices, np.int64)
    jj = np.asarray(j_indices, np.int64)

    # U tables: [union(33) | h_tgt(33)] per action
    U = np.zeros((NA, 66), np.float32)
    U[np.arange(NA), Pa] = 1.0
    U[np.arange(NA), Ma] = 1.0
    U[np.arange(NA), Ta] = 1.0
    U[np.arange(NA), NB + Ta] = 1.0
    import ml_dtypes
    u1 = U[:P].astype(ml_dtypes.bfloat16)
    u2 = (U[P + (np.arange(P) % 4)] / 32.0).astype(ml_dtypes.bfloat16)

    rhs2 = np.zeros((NB + 1, NA + 49), np.float32)
    # count[a] = 2 + 3*oob[a] - np[P[a]] - np[M[a]] + np[T[a]]
    np.add.at(rhs2, (Pa, np.arange(NA)), -1.0)
    np.add.at(rhs2, (Ma, np.arange(NA)), -1.0)
    np.add.at(rhs2, (Ta, np.arange(NA)), 1.0)
    rhs2[NB, :NA] = 2.0 + 3.0 * oob.astype(np.float32)
    cell = 7 * ii + jj
    rhs2[np.arange(NB), NA + cell] = 1.0
    import ml_dtypes
    rhs2 = rhs2.astype(ml_dtypes.bfloat16)

    acst = np.zeros((P, 98), np.float32)
    bcst = np.zeros((P, 98), np.float32)
    acst[:, 0::2] = 1.0
    acst[:, 1::2] = -1.0
    bcst[:, 1::2] = 1.0
    import ml_dtypes
    e8 = np.zeros((8, 8 * P), ml_dtypes.bfloat16)
    for qq in range(8):
        e8[qq, qq * P : (qq + 1) * P] = 1.0
    mcol = np.zeros((P, 2), np.float32)
    mcol[:, 0] = np.arange(P)
    mcol[:, 1] = 128 + (np.arange(P) % 4)
    return u1, u2, rhs2, acst, bcst, e8, mcol


def make_in_map(inputs, lo, hi):
    """Build one core's in_map from full unsharded inputs dict."""
    u1, u2, rhs2, acst, bcst, e8, mcol = _consts(
        inputs["action_pos_ids"], inputs["action_mid_indices"],
        inputs["action_target_indices"], inputs["oob_mask"],
        inputs["i_indices"], inputs["j_indices"])
    s = slice(lo, hi)
    return {
        "pegs": np.ascontiguousarray(np.asarray(inputs["pegs"][s], np.float32)),
        "n_pegs": np.ascontiguousarray(np.asarray(inputs["n_pegs"][s], np.int32)),
        "done": np.ascontiguousarray(np.asarray(inputs["done"][s]).astype(np.uint8)),
        "total_reward": np.ascontiguousarray(np.asarray(inputs["total_reward"][s], np.float32)),
        "actions": np.ascontiguousarray(np.asarray(inputs["actions"][s], np.int32)),
        "u1": u1, "u2": u2, "rhs2": rhs2, "acst": acst, "bcst": bcst,
        "e8": e8, "mcol": mcol,
    }


def kernel(pegs, n_pegs, done, total_reward, actions,
           action_pos_ids, action_mid_indices, action_target_indices,
           oob_mask, i_indices, j_indices):
    from concourse.bass_utils import run_bass_kernel_spmd

    if "nc" not in _CACHE:
        _CACHE["nc"] = _build_bass()
    nc = _CACHE["nc"]

    u1, u2, rhs2, acst, bcst, e8, mcol = _consts(
        action_pos_ids, action_mid_indices, action_target_indices,
        oob_mask, i_indices, j_indices)

    pegs = np.ascontiguousarray(np.asarray(pegs, np.float32))
    n_pegs = np.ascontiguousarray(np.asarray(n_pegs, np.int32))
    done_u = np.ascontiguousarray(np.asarray(done).astype(np.uint8))
    trew = np.ascontiguousarray(np.asarray(total_reward, np.float32))
    actions = np.ascontiguousarray(np.asarray(actions, np.int32))

    in_maps = []
    for c in range(N_CORES):
        s = slice(c * E, (c + 1) * E)
        in_maps.append({
            "pegs": pegs[s],
            "n_pegs": n_pegs[s],
            "done": done_u[s],
            "total_reward": trew[s],
            "actions": actions[s],
            "u1": u1, "u2": u2,
            "rhs2": rhs2,
            "acst": acst,
            "bcst": bcst,
            "e8": e8, "mcol": mcol,
        })

    res = run_bass_kernel_spmd(nc, in_maps, core_ids=list(range(N_CORES)))

    rewards = np.concatenate([res.results[c]["rewards_o"] for c in range(N_CORES)])
    states = np.concatenate([res.results[c]["states_o"] for c in range(N_CORES)])
    done_new = np.concatenate([res.results[c]["done_o"] for c in range(N_CORES)])
    pegs_new = np.concatenate(
        [np.asarray(res.results[c]["pegs_o"]).astype(np.float32) for c in range(N_CORES)]
    )
    npegs_new = np.concatenate([res.results[c]["npegs_o"] for c in range(N_CORES)])
    trew_new = np.concatenate([res.results[c]["trew_o"] for c in range(N_CORES)])

    n = N_ENV
    return (
        rewards.astype(np.float32),
        states.reshape(n, 7, 7, 3).astype(np.float32),
        done_new.astype(bool),
        pegs_new.astype(np.float32),
        npegs_new.astype(np.int32),
        trew_new.astype(np.float32),
    )
